# revision 1
# baseline (speedup 1.0000x reference)
"""Trainium2 Bass kernel: BlockAttnRes forward.

Reference computation (per batch b, position t):
    k[n]   = s[n] / sqrt(mean(s[n]^2) + eps)        n in [0, 9)
    score  = k[n] . w                                (w = queries[layer_idx])
    alpha  = softmax(score over n)
    h[t]   = sum_n alpha[n] * s[n]                   (d = 512)

Distribution: batch dim B=8 -> one batch per NeuronCore (8 cores), no
cross-core communication.  Per core: T=4096 positions processed in 32
tiles of 128 (partition dim = position).

Per-tile engine assignment (measured ~8.3us/tile steady state, all of
DVE/ACT/DMA saturated):
    DMA   : load [128, 9, 512] f32 as 3 chunk DMAs (contiguous 6KB rows)
    ACT   : ssq[n]  = sum_d s^2      (Square + accum_out, 9 ops)
            rsq     = Exp(-0.5 * Ln(ssq/512 + eps))   (= 1/rms)
            e       = Exp(score - max)
    DVE   : dot[n]  = sum_d s*w      (scalar_tensor_tensor + accum, 9 ops)
            score = dot*rsq; -max; sum_e; 1/sum_e; alpha = e/sum_e
            diag blocks for n<8: ONE broadcast tensor_tensor I x alpha
            h = alpha_8*s_8 + h_psum (STT, doubles as PSUM->SBUF move)
    PE    : h_psum += diag(alpha_n).T @ s_n  (8 accumulating fp32 matmuls)

All ACT functions (square, ln, exp) live in the single
`natural_log_exp_and_others` table set -> one ACT_TABLE_LOAD total
(pinned via PinnedBacc below; the stock chooser thrashes sets).
"""

import numpy as np

B, T, N, D = 8, 4096, 9, 512
P = 128
EPS = 1e-6
NCORES = 8

_CACHE = {}


def _build_bass(t_len=T, wsum_dtype="float32", n_chunks=3):
    import concourse.bass as bass
    import concourse.tile as tile
    from concourse import bacc, mybir

    f32 = mybir.dt.float32
    Alu = mybir.AluOpType
    Act = mybir.ActivationFunctionType
    Ax = mybir.AxisListType
    mm_dt = getattr(mybir.dt, wsum_dtype)

    ntiles = t_len // P

    # Bacc (not raw Bass): its compile() pass splits multi-sem waits into
    # InstEventSemaphore — TRN2 engine instructions hold at most ONE wait.
    #
    # Pin the ACT table set: all activation funcs used here (square, ln,
    # exp, copy) live in `natural_log_exp_and_others`; the stock greedy
    # chooser ping-pongs between sets (~2 table loads/tile = 83us of pure
    # ACT stall).  Emptying every other set (positions preserved — the
    # index is the act_func_set_id) forces a single load.
    PINNED_SET = "natural_log_exp_and_others"

    class PinnedBacc(bacc.Bacc):
        def insert_act_table_loads(self):
            import bass_rust as _bass_rust
            from concourse.hw_specs import get_activation_tables

            all_tables = get_activation_tables(self.m.arch)
            used = {
                i.func
                for b in self.main_func.blocks
                for i in b.instructions
                if isinstance(i, mybir.InstActivation)
            }
            if used and PINNED_SET in all_tables and used <= all_tables[PINNED_SET]:
                tables = [
                    (name, funcs if name == PINNED_SET else set())
                    for name, funcs in all_tables.items()
                ]
            else:
                tables = list(all_tables.items())
            _bass_rust.insert_act_table_loads(self, tables)

    nc = PinnedBacc("TRN2", target_bir_lowering=False, debug=False)
    src = nc.dram_tensor("src", [t_len, N, D], f32, kind="ExternalInput").ap()
    wq = nc.dram_tensor("wq", [P, D], f32, kind="ExternalInput").ap()
    idn = nc.dram_tensor("idn", [P, P], f32, kind="ExternalInput").ap()
    out = nc.dram_tensor("out", [t_len, D], f32, kind="ExternalOutput").ap()

    src_t = src.rearrange("(c p) n d -> c p n d", p=P)
    out_t = out.rearrange("(c p) d -> c p d", p=P)

    with tile.TileContext(nc) as tc:
        with (
            tc.tile_pool(name="const", bufs=1) as const_pool,
            tc.tile_pool(name="srcp", bufs=6) as src_pool,
            tc.tile_pool(name="scratch", bufs=6) as scr_pool,
            tc.tile_pool(name="small", bufs=8) as small_pool,
            tc.tile_pool(name="diag", bufs=5) as diag_pool,
            tc.tile_pool(name="hout", bufs=6) as out_pool,
            tc.tile_pool(name="psum", bufs=8, space="PSUM") as psum_pool,
        ):
            w_sb = const_pool.tile([P, D], f32)
            nc.sync.dma_start(out=w_sb, in_=wq)
            i_sb = const_pool.tile([P, P], f32)
            nc.sync.dma_start(out=i_sb, in_=idn)
            eps_sb = const_pool.tile([P, 1], f32)
            nc.vector.memset(eps_sb, EPS)

            assert N % n_chunks == 0
            cn = N // n_chunks  # n's per chunk

            for c in range(ntiles):
                # Load sources in n_chunks separate DMAs (separate tiles) so
                # each consumer instruction depends on exactly one DMA
                # semaphore (hardware allows ONE wait per instruction).
                chunks = []
                for k in range(n_chunks):
                    sk = src_pool.tile([P, cn, D], mm_dt, tag=f"s{k}")
                    src_in = src_t[c, :, k * cn : (k + 1) * cn, :]
                    if mm_dt != f32:
                        src_in = src_in.bitcast(mm_dt)
                    nc.sync.dma_start(out=sk, in_=src_in)
                    chunks.append(sk)

                def s_mm(n):
                    return chunks[n // cn][:, n % cn, :]

                def s_of(n):
                    v = s_mm(n)
                    return v.bitcast(f32) if mm_dt != f32 else v

                # ssq[t, n] = sum_d s^2   (ACT)
                ssq = small_pool.tile([P, N], f32, tag="ssq")
                sq = scr_pool.tile([P, D], f32, tag="sq")
                for n in range(N):
                    nc.scalar.activation(
                        out=sq,
                        in_=s_of(n),
                        func=Act.Square,
                        accum_out=ssq[:, n : n + 1],
                    )

                # dot[t, n] = sum_d s*w   (DVE)
                dot = small_pool.tile([P, N], f32, tag="dot")
                prod = scr_pool.tile([P, D], f32, tag="prod")
                for n in range(N):
                    nc.vector.scalar_tensor_tensor(
                        out=prod,
                        in0=s_of(n),
                        scalar=0.0,
                        in1=w_sb,
                        op0=Alu.bypass,
                        op1=Alu.mult,
                        accum_out=dot[:, n : n + 1],
                    )

                # rsq = (ssq/D + eps)^(-1/2) via Exp(-0.5 * Ln(x))
                rsq = small_pool.tile([P, N], f32, tag="rsq")
                nc.scalar.activation(
                    out=rsq, in_=ssq, func=Act.Ln, scale=1.0 / D, bias=eps_sb
                )
                nc.scalar.activation(out=rsq, in_=rsq, func=Act.Exp, scale=-0.5)

                score = small_pool.tile([P, N], f32, tag="score")
                nc.vector.tensor_mul(score, dot, rsq)

                nmx = small_pool.tile([P, 1], f32, tag="nmx")
                nc.vector.tensor_reduce(
                    out=nmx, in_=score, axis=Ax.X, op=Alu.max, negate=True
                )
                e = small_pool.tile([P, N], f32, tag="e")
                nc.scalar.activation(out=e, in_=score, func=Act.Exp, bias=nmx)
                sume = small_pool.tile([P, 1], f32, tag="sume")
                nc.vector.tensor_reduce(out=sume, in_=e, axis=Ax.X, op=Alu.add)
                rs = small_pool.tile([P, 1], f32, tag="rs")
                nc.vector.reciprocal(out=rs, in_=sume)
                # alpha = e * (1/sum_e) — normalizing up front lets the final
                # PSUM->SBUF move be a plain fused multiply-add (STT) that
                # also absorbs the last n's contribution.
                al = small_pool.tile([P, N], f32, tag="al")
                nc.vector.tensor_scalar_mul(al, e, rs)

                # diag(e_n) = I * e[:, n] — ONE broadcast tensor_tensor:
                # in0 = I broadcast over n (stride-0), in1 = e broadcast over
                # the 128 columns (stride-0 innermost).
                dg = diag_pool.tile([P, N, P], mm_dt, tag="dg")
                i_b = bass.AP(
                    tensor=i_sb.tensor,
                    offset=i_sb.offset,
                    ap=[i_sb.ap[0], [0, N - 1], i_sb.ap[1]],
                )
                a_b = bass.AP(
                    tensor=al.tensor,
                    offset=al.offset,
                    ap=[al.ap[0], [al.ap[1][0], N - 1], [0, P]],
                )
                nc.vector.tensor_mul(dg[:, : N - 1, :], i_b, a_b)

                # h_psum += diag(e_n).T @ s_n
                hp = psum_pool.tile([P, D], f32, tag="hp")
                for n in range(N - 1):
                    nc.tensor.matmul(
                        hp,
                        dg[:, n, :],
                        s_mm(n),
                        start=(n == 0),
                        stop=(n == N - 2),
                    )

                # h = alpha_8 * s_8 + h_psum  (one STT, PSUM src; doubles as
                # the PSUM -> SBUF move and the last n's accumulation)
                hs = out_pool.tile([P, D], f32, tag="hs")
                nc.vector.scalar_tensor_tensor(
                    out=hs,
                    in0=s_of(N - 1),
                    scalar=al[:, N - 1 : N],
                    in1=hp,
                    op0=Alu.mult,
                    op1=Alu.add,
                )
                nc.sync.dma_start(out=out_t[c], in_=hs)

    nc.compile()
    return nc


def _get_nc(t_len=T, wsum_dtype="float32"):
    key = (t_len, wsum_dtype)
    if key not in _CACHE:
        _CACHE[key] = _build_bass(t_len, wsum_dtype)
    return _CACHE[key]


def _make_in_maps(sources, queries, layer_idx):
    sources = np.ascontiguousarray(np.asarray(sources, dtype=np.float32))
    queries = np.asarray(queries, dtype=np.float32)
    w = queries[int(layer_idx)]
    w_rep = np.ascontiguousarray(np.broadcast_to(w[None, :], (P, D)).astype(np.float32))
    idn = np.eye(P, dtype=np.float32)
    return [
        {"src": np.ascontiguousarray(sources[b]), "wq": w_rep, "idn": idn}
        for b in range(sources.shape[0])
    ]


def kernel(sources, queries, layer_idx):
    from concourse.bass_utils import run_bass_kernel_spmd

    nc = _get_nc()
    in_maps = _make_in_maps(sources, queries, layer_idx)
    res = run_bass_kernel_spmd(nc, in_maps, core_ids=list(range(NCORES)))
    return np.stack([res.results[b]["out"] for b in range(NCORES)], axis=0)



# revision 2
# speedup vs baseline: 1.2224x; 1.2224x over previous
"""Trainium2 Bass kernel: BlockAttnRes forward.

Reference computation (per batch b, position t):
    k[n]   = s[n] / sqrt(mean(s[n]^2) + eps)        n in [0, 9)
    score  = k[n] . w                                (w = queries[layer_idx])
    alpha  = softmax(score over n)
    h[t]   = sum_n alpha[n] * s[n]                   (d = 512)

Distribution: batch dim B=8 -> one batch per NeuronCore (8 cores), no
cross-core communication.  Per core: T=4096 positions processed in 32
tiles of 128 (partition dim = position).

Per-tile engine assignment (measured ~8.3us/tile steady state, all of
DVE/ACT/DMA saturated):
    DMA   : load [128, 9, 512] f32 as 3 chunk DMAs (contiguous 6KB rows)
    ACT   : ssq[n]  = sum_d s^2      (Square + accum_out, 9 ops)
            rsq     = Exp(-0.5 * Ln(ssq/512 + eps))   (= 1/rms)
            e       = Exp(score - max)
    DVE   : dot[n]  = sum_d s*w      (scalar_tensor_tensor + accum, 9 ops)
            score = dot*rsq; -max; sum_e; 1/sum_e; alpha = e/sum_e
            diag blocks for n<8: ONE broadcast tensor_tensor I x alpha
            h = alpha_8*s_8 + h_psum (STT, doubles as PSUM->SBUF move)
    PE    : h_psum += diag(alpha_n).T @ s_n  (8 accumulating fp32 matmuls)

All ACT functions (square, ln, exp) live in the single
`natural_log_exp_and_others` table set -> one ACT_TABLE_LOAD total
(pinned via PinnedBacc below; the stock chooser thrashes sets).
"""

import numpy as np

B, T, N, D = 8, 4096, 9, 512
P = 128
EPS = 1e-6
NCORES = 8

_CACHE = {}


def _build_bass(t_len=T, wsum_dtype="float32", n_chunks=3):
    import concourse.bass as bass
    import concourse.tile as tile
    from concourse import bacc, mybir

    f32 = mybir.dt.float32
    Alu = mybir.AluOpType
    Act = mybir.ActivationFunctionType
    Ax = mybir.AxisListType
    mm_dt = getattr(mybir.dt, wsum_dtype)

    ntiles = t_len // P

    # Bacc (not raw Bass): its compile() pass splits multi-sem waits into
    # InstEventSemaphore — TRN2 engine instructions hold at most ONE wait.
    #
    # Pin the ACT table set: all activation funcs used here (square, ln,
    # exp, copy) live in `natural_log_exp_and_others`; the stock greedy
    # chooser ping-pongs between sets (~2 table loads/tile = 83us of pure
    # ACT stall).  Emptying every other set (positions preserved — the
    # index is the act_func_set_id) forces a single load.
    PINNED_SET = "natural_log_exp_and_others"

    class PinnedBacc(bacc.Bacc):
        def insert_act_table_loads(self):
            import bass_rust as _bass_rust
            from concourse.hw_specs import get_activation_tables

            all_tables = get_activation_tables(self.m.arch)
            used = {
                i.func
                for b in self.main_func.blocks
                for i in b.instructions
                if isinstance(i, mybir.InstActivation)
            }
            if used and PINNED_SET in all_tables and used <= all_tables[PINNED_SET]:
                tables = [
                    (name, funcs if name == PINNED_SET else set())
                    for name, funcs in all_tables.items()
                ]
            else:
                tables = list(all_tables.items())
            _bass_rust.insert_act_table_loads(self, tables)

    nc = PinnedBacc("TRN2", target_bir_lowering=False, debug=False)
    src = nc.dram_tensor("src", [t_len, N, D], f32, kind="ExternalInput").ap()
    wq = nc.dram_tensor("wq", [P, D], f32, kind="ExternalInput").ap()
    idn = nc.dram_tensor("idn", [P, P], f32, kind="ExternalInput").ap()
    out = nc.dram_tensor("out", [t_len, D], f32, kind="ExternalOutput").ap()

    src_t = src.rearrange("(c p) n d -> c p n d", p=P)
    out_t = out.rearrange("(c p) d -> c p d", p=P)

    with tile.TileContext(nc) as tc:
        with (
            tc.tile_pool(name="const", bufs=1) as const_pool,
            tc.tile_pool(name="srcp", bufs=6) as src_pool,
            tc.tile_pool(name="scratch", bufs=6) as scr_pool,
            tc.tile_pool(name="small", bufs=8) as small_pool,
            tc.tile_pool(name="diag", bufs=5) as diag_pool,
            tc.tile_pool(name="hout", bufs=6) as out_pool,
            tc.tile_pool(name="psum", bufs=8, space="PSUM") as psum_pool,
        ):
            w_sb = const_pool.tile([P, D], f32)
            nc.sync.dma_start(out=w_sb, in_=wq)
            i_sb = const_pool.tile([P, P], f32)
            nc.sync.dma_start(out=i_sb, in_=idn)
            eps_sb = const_pool.tile([P, 1], f32)
            nc.vector.memset(eps_sb, EPS)

            assert N % n_chunks == 0
            cn = N // n_chunks  # n's per chunk

            for c in range(ntiles):
                # Load sources in n_chunks separate DMAs (separate tiles) so
                # each consumer instruction depends on exactly one DMA
                # semaphore (hardware allows ONE wait per instruction).
                chunks = []
                for k in range(n_chunks):
                    sk = src_pool.tile([P, cn, D], mm_dt, tag=f"s{k}")
                    src_in = src_t[c, :, k * cn : (k + 1) * cn, :]
                    if mm_dt != f32:
                        src_in = src_in.bitcast(mm_dt)
                    nc.sync.dma_start(out=sk, in_=src_in)
                    chunks.append(sk)

                def s_mm(n):
                    return chunks[n // cn][:, n % cn, :]

                def s_of(n):
                    v = s_mm(n)
                    return v.bitcast(f32) if mm_dt != f32 else v

                # ssq[t, n] = sum_d s^2   (ACT)
                ssq = small_pool.tile([P, N], f32, tag="ssq")
                sq = scr_pool.tile([P, D], f32, tag="sq")
                for n in range(N):
                    nc.scalar.activation(
                        out=sq,
                        in_=s_of(n),
                        func=Act.Square,
                        accum_out=ssq[:, n : n + 1],
                    )

                # dot[t, n] = sum_d s*w   (DVE)
                dot = small_pool.tile([P, N], f32, tag="dot")
                prod = scr_pool.tile([P, D], f32, tag="prod")
                for n in range(N):
                    nc.vector.scalar_tensor_tensor(
                        out=prod,
                        in0=s_of(n),
                        scalar=0.0,
                        in1=w_sb,
                        op0=Alu.bypass,
                        op1=Alu.mult,
                        accum_out=dot[:, n : n + 1],
                    )

                # rsq = (ssq/D + eps)^(-1/2) via Exp(-0.5 * Ln(x))
                rsq = small_pool.tile([P, N], f32, tag="rsq")
                nc.scalar.activation(
                    out=rsq, in_=ssq, func=Act.Ln, scale=1.0 / D, bias=eps_sb
                )
                nc.scalar.activation(out=rsq, in_=rsq, func=Act.Exp, scale=-0.5)

                score = small_pool.tile([P, N], f32, tag="score")
                nc.vector.tensor_mul(score, dot, rsq)

                nmx = small_pool.tile([P, 1], f32, tag="nmx")
                nc.vector.tensor_reduce(
                    out=nmx, in_=score, axis=Ax.X, op=Alu.max, negate=True
                )
                e = small_pool.tile([P, N], f32, tag="e")
                nc.scalar.activation(out=e, in_=score, func=Act.Exp, bias=nmx)
                sume = small_pool.tile([P, 1], f32, tag="sume")
                nc.vector.tensor_reduce(out=sume, in_=e, axis=Ax.X, op=Alu.add)
                rs = small_pool.tile([P, 1], f32, tag="rs")
                nc.vector.reciprocal(out=rs, in_=sume)
                # alpha = e * (1/sum_e) — normalizing up front lets the final
                # PSUM->SBUF move be a plain fused multiply-add (STT) that
                # also absorbs the last n's contribution.
                al = small_pool.tile([P, N], f32, tag="al")
                nc.vector.tensor_scalar_mul(al, e, rs)

                # diag(e_n) = I * e[:, n] — ONE broadcast tensor_tensor:
                # in0 = I broadcast over n (stride-0), in1 = e broadcast over
                # the 128 columns (stride-0 innermost).
                dg = diag_pool.tile([P, N, P], mm_dt, tag="dg")
                i_b = bass.AP(
                    tensor=i_sb.tensor,
                    offset=i_sb.offset,
                    ap=[i_sb.ap[0], [0, N - 1], i_sb.ap[1]],
                )
                a_b = bass.AP(
                    tensor=al.tensor,
                    offset=al.offset,
                    ap=[al.ap[0], [al.ap[1][0], N - 1], [0, P]],
                )
                nc.vector.tensor_mul(dg[:, : N - 1, :], i_b, a_b)

                # h_psum += diag(e_n).T @ s_n
                hp = psum_pool.tile([P, D], f32, tag="hp")
                for n in range(N - 1):
                    nc.tensor.matmul(
                        hp,
                        dg[:, n, :],
                        s_mm(n),
                        start=(n == 0),
                        stop=(n == N - 2),
                    )

                # h = alpha_8 * s_8 + h_psum  (one STT, PSUM src; doubles as
                # the PSUM -> SBUF move and the last n's accumulation)
                hs = out_pool.tile([P, D], f32, tag="hs")
                nc.vector.scalar_tensor_tensor(
                    out=hs,
                    in0=s_of(N - 1),
                    scalar=al[:, N - 1 : N],
                    in1=hp,
                    op0=Alu.mult,
                    op1=Alu.add,
                )
                nc.sync.dma_start(out=out_t[c], in_=hs)

    nc.compile()
    return nc


def _get_nc(t_len=T, wsum_dtype="float32r"):
    key = (t_len, wsum_dtype)
    if key not in _CACHE:
        _CACHE[key] = _build_bass(t_len, wsum_dtype)
    return _CACHE[key]


def _make_in_maps(sources, queries, layer_idx):
    sources = np.ascontiguousarray(np.asarray(sources, dtype=np.float32))
    queries = np.asarray(queries, dtype=np.float32)
    w = queries[int(layer_idx)]
    w_rep = np.ascontiguousarray(np.broadcast_to(w[None, :], (P, D)).astype(np.float32))
    idn = np.eye(P, dtype=np.float32)
    return [
        {"src": np.ascontiguousarray(sources[b]), "wq": w_rep, "idn": idn}
        for b in range(sources.shape[0])
    ]


def kernel(sources, queries, layer_idx):
    from concourse.bass_utils import run_bass_kernel_spmd

    nc = _get_nc()
    in_maps = _make_in_maps(sources, queries, layer_idx)
    res = run_bass_kernel_spmd(nc, in_maps, core_ids=list(range(NCORES)))
    return np.stack([res.results[b]["out"] for b in range(NCORES)], axis=0)

